# revision 61
# baseline (speedup 1.0000x reference)
"""Trainium2 Bass kernel for nn_DAWN_10419590660472 (moe_routing transformer).

Sharding: 8 cores = 4 batches x 2 vocab-halves. Each core computes the full
4-layer body for its batch (the causal residual stream cannot be
sequence-split without communication), then the tied-embedding head for all
1024 tokens over its 16000-entry vocab half. All cores run the SAME program;
only input data differs. Heavy matmuls in bf16 with fp32 PSUM accumulation.

Host-folded math:
- attn.mean(-1) == 1/S exactly (softmax rows sum to 1), so the routing gate
  sigmoid(ctx) is a per-layer constant folded into the sem projection.
- LN affine params fold into downstream weights (identity here: s=1, b=0).
- top_k(fin, 8): nc.vector.max yields the 8 largest per token; token_recipe
  = masked_softmax(fin) @ rec_sm, a matmul (no gather).
- Attention softmax needs no max-subtraction (scores are O(1)); denominators
  come free from a ones-augmented value matrix in the AV matmul and are
  folded into the PSUM->SBUF copy as a reciprocal multiply.
"""

import numpy as np
import ml_dtypes

VOC = 32000; D = 512; DFF = 2048; L = 4; H = 8; DH = D // H
NN = 256; NB = 32; R = 128; B = 4; S = 1024
NCORES = 8
TT = S // 128          # 8 token tiles
DS = D // 128          # 4 d-slices
FS = DFF // 128        # 16 dff-slices
QC = S // 512          # 2 query chunks
NG = NB // 4           # 8 groups of 4 basis matrices
VH = VOC // 2          # vocab half per core
VCH = 500              # head vocab chunk (<=512)
NVC = VH // VCH        # 32
VG = 8                 # emb streaming groups
VPG = NVC // VG        # 4 chunks per group
EPS = 1e-5

BF16 = ml_dtypes.bfloat16
_cache = {}
DEBUG = False


def _softmax_np(x, axis=-1):
    m = x.max(axis=axis, keepdims=True)
    e = np.exp(x - m)
    return e / e.sum(axis=axis, keepdims=True)


def _preprocess(inputs):
    f32 = lambda k: np.asarray(inputs[k], dtype=np.float32)
    ids = np.asarray(inputs["input_ids"])
    token_emb = f32("token_emb"); pos_emb = f32("pos_emb")
    basis_A = f32("basis_A"); basis_emb = f32("basis_emb")
    q_w = f32("q_w"); k_w = f32("k_w"); ao_w = f32("ao_w")
    recipe = f32("recipe"); ctx_pat = f32("ctx_pat")
    vout_w = f32("vout_w"); up_w = f32("up_w"); down_w = f32("down_w")
    ln1_s = f32("ln1_s"); ln2_s = f32("ln2_s"); lnf_s = f32("lnf_s")

    for k in ("q_b", "k_b", "ao_b", "vout_b", "up_b", "down_b",
              "ln1_b", "ln2_b", "lnf_b"):
        assert not np.any(np.asarray(inputs[k])), f"nonzero {k} unsupported"

    scale = 1.0 / np.sqrt(DH)
    x0 = token_emb[ids] + pos_emb[:S][None]              # [B, S, D]

    def part_first(a, nslice):
        # [nslice*128, F] -> [128, nslice, F]
        return np.ascontiguousarray(
            a.reshape(nslice, 128, -1).transpose(1, 0, 2))

    wq = np.empty((L, 128, DS, D), dtype=BF16)
    wk = np.empty((L, 128, DS, D), dtype=BF16)
    wao = np.empty((L, 128, DS, D), dtype=BF16)
    gT = np.empty((L, 128, DS, NN), dtype=BF16)
    recT = np.empty((L, 128, 2, NB), dtype=BF16)
    a_cat = np.empty((L, 128, DS, NB * R), dtype=BF16)
    wvout = np.empty((L, 128, D), dtype=BF16)
    wup = np.empty((L, 128, DS, DFF), dtype=BF16)
    wdn = np.empty((L, 128, FS, D), dtype=BF16)

    for l in range(L):
        wq[l] = part_first((q_w[l] * ln1_s[l][None, :] * scale).T, DS)
        wk[l] = part_first((k_w[l] * ln1_s[l][None, :]).T, DS)
        wao[l] = part_first(ao_w[l].T, DS)
        rs = _softmax_np(recipe[l])                      # [NN, NB]
        emb_sem = rs @ basis_emb                         # [NN, D]
        gate = 1.0 / (1.0 + np.exp(-(ctx_pat[l].sum(-1) / S)))
        gT[l] = part_first(((emb_sem * ln1_s[l][None, :]) * gate[:, None]).T, DS)
        recT[l] = part_first(rs, 2)
        ae = basis_A * ln1_s[l][None, :, None]           # [NB, D, R]
        a_cat[l] = part_first(ae.transpose(1, 0, 2).reshape(D, NB * R), DS)
        wvout[l] = vout_w[l].T.astype(BF16)              # [R, D]
        wup[l] = part_first((up_w[l] * ln2_s[l][None, :]).T, DS)
        wdn[l] = part_first(down_w[l].T, FS)

    eT_full = part_first((token_emb * lnf_s[None, :]).T, DS).astype(BF16)
    ident = np.eye(128, dtype=BF16)
    # head h's ones-row lives at partition 32*(h//4) + h%4 so that per-quad
    # reciprocal/broadcast ops start at 32-aligned partitions
    sel = np.zeros((128, H * 64), dtype=BF16)
    for h in range(H):
        sel[32 * (h // 4) + h % 4, h * 64:(h + 1) * 64] = 1

    shared = dict(wq=wq, wk=wk, wao=wao, gT=gT, recT=recT, a_cat=a_cat,
                  wvout=wvout, wup=wup, wdn=wdn, ident=ident, sel=sel)
    per_core = []
    for c in range(NCORES):
        b, half = c // 2, c % 2
        m = dict(shared)
        m["x0"] = np.ascontiguousarray(x0[b]).astype(np.float32)
        m["eT"] = np.ascontiguousarray(eT_full[:, :, half * VH:(half + 1) * VH])
        per_core.append(m)
    return per_core


def _build_nc():
    import concourse.mybir as mybir
    import concourse.tile as tile
    from concourse import bacc
    from concourse.alu_op_type import AluOpType as Alu

    AF = mybir.ActivationFunctionType
    bf = mybir.dt.bfloat16
    f32 = mybir.dt.float32

    nc = bacc.Bacc("TRN2", target_bir_lowering=False, debug=False,
                   num_devices=NCORES)

    din = lambda n, shp, dt=bf: nc.dram_tensor(n, shp, dt, kind="ExternalInput")
    dr = dict(
        x0=din("x0", [S, D], f32),
        wq=din("wq", [L, 128, DS, D]), wk=din("wk", [L, 128, DS, D]),
        wao=din("wao", [L, 128, DS, D]), gT=din("gT", [L, 128, DS, NN]),
        recT=din("recT", [L, 128, 2, NB]),
        a_cat=din("a_cat", [L, 128, DS, NB * R]),
        wvout=din("wvout", [L, 128, D]), wup=din("wup", [L, 128, DS, DFF]),
        wdn=din("wdn", [L, 128, FS, D]), eT=din("eT", [128, DS, VH]),
        ident=din("ident", [128, 128]), sel=din("sel", [128, H * 64]),
        out=nc.dram_tensor("logits", [S, VH], bf, kind="ExternalOutput"),
    )

    with tile.TileContext(nc) as tc:
        _emit(nc, tc, mybir, Alu, AF, bf, f32, dr)

    nc.compile()
    return nc


def _emit(nc, tc, mybir, Alu, AF, bf, f32, dr):
    from contextlib import ExitStack
    ctx = ExitStack()
    pool = lambda name, bufs, space="SBUF": ctx.enter_context(
        tc.tile_pool(name=name, bufs=bufs, space=space))

    P_x = pool("x", 1)
    P_const = pool("const", 1)
    P_w = pool("w", 2)                 # small per-layer weights (double-buffered)
    P_ac = pool("ac", 2)               # a_cat halves
    P_big = pool("big", 2)             # wup / wdn / emb chunks
    P_act = pool("act", 1)             # per-layer activations
    P_nrm = pool("nrm", 1)             # token-major LN outputs (transient)
    P_attn = pool("attn", 3)           # eT buffers (2-ahead score pipeline)
    P_rt = pool("rt", 2)               # routing temporaries
    P_sm = pool("sm", 2)               # small stats tiles
    P_hd = pool("hd", 6)               # head staging
    P_ps = pool("ps", 2, "PSUM")       # generic matmul psum
    P_psA = pool("psA", 3, "PSUM")     # xA / AV psum
    P_psS = pool("psS", 3, "PSUM")     # scores/transpose psum + denom bcast

    ident = P_const.tile([128, 128], bf)
    nc.sync.dma_start(out=ident, in_=dr["ident"][:, :])
    eps_sb = P_const.tile([128, 1], f32)
    nc.vector.memset(eps_sb, EPS)
    # sel[:, h*64:(h+1)*64] is all-ones on partition h, zero elsewhere:
    # matmul(lhsT=sel_h, rhs=rec8[0:8]) broadcasts rec8 row h to 64 partitions
    sel = P_const.tile([128, H * 64], bf)
    nc.sync.dma_start(out=sel, in_=dr["sel"][:, :])

    x_sb = P_x.tile([128, TT, D], f32)
    for t in range(TT):
        nc.sync.dma_start(out=x_sb[:, t, :],
                          in_=dr["x0"][t * 128:(t + 1) * 128, :])

    def ln_pass(dstT, mid=None):
        # batched stats for all 8 token tiles -> one sqrt + one reciprocal
        mv8 = P_sm.tile([128, TT, 2], f32, tag="mv8")
        for t in range(TT):
            stats = P_sm.tile([128, 6], f32, tag="st")
            nc.vector.bn_stats(out=stats, in_=x_sb[:, t, :])
            nc.vector.bn_aggr(out=mv8[:, t, :], in_=stats)
        rstd8 = P_sm.tile([128, TT], f32, tag="rs8")
        nc.scalar.activation(out=rstd8, in_=mv8[:, :, 1], func=AF.Sqrt,
                             bias=eps_sb)
        nc.vector.reciprocal(out=rstd8, in_=rstd8)
        for t in range(TT):
            nrm = P_nrm.tile([128, D], bf, tag="nrm")
            nc.vector.tensor_scalar(out=nrm, in0=x_sb[:, t, :],
                                    scalar1=mv8[:, t, 0:1],
                                    scalar2=rstd8[:, t:t + 1],
                                    op0=Alu.subtract, op1=Alu.mult)
            for ds in range(DS):
                transpose128(dstT[:, ds, t * 128:(t + 1) * 128],
                             nrm[:, ds * 128:(ds + 1) * 128],
                             cp=(nc.vector if ds % 2 == 0 else nc.scalar))
            if t == 3 and mid is not None:
                mid()   # interleave PE-heavy work behind tiles 4-7

    def transpose128(dst_sb, src_sb, cp=None):
        ps = P_psS.tile([128, 128], bf, tag="psS")
        nc.tensor.transpose(ps, src_sb, ident)
        if cp is nc.scalar:
            nc.scalar.copy(out=dst_sb, in_=ps)
        else:
            nc.vector.tensor_copy(out=dst_sb, in_=ps)

    for l in range(L):
        wq_l = P_w.tile([128, DS, D], bf, tag="wq", bufs=1)
        wk_l = P_w.tile([128, DS, D], bf, tag="wk", bufs=1)
        wao_l = P_w.tile([128, DS, D], bf, tag="wao", bufs=1)
        g_l = P_w.tile([128, DS, NN], bf, tag="g", bufs=1)
        rec_l = P_w.tile([128, 2, NB], bf, tag="rec")
        wv_l = P_w.tile([128, D], bf, tag="wv", bufs=1)
        nc.sync.dma_start(out=wq_l, in_=dr["wq"][l])
        nc.sync.dma_start(out=wk_l, in_=dr["wk"][l])
        nc.sync.dma_start(out=wao_l, in_=dr["wao"][l])
        nc.sync.dma_start(out=g_l, in_=dr["gT"][l])
        nc.sync.dma_start(out=rec_l, in_=dr["recT"][l])
        nc.sync.dma_start(out=wv_l, in_=dr["wvout"][l])

        nrmT = P_act.tile([128, DS, S], bf, tag="nrmT")
        qT = P_act.tile([128, DS, S], bf, tag="qT")  # slot reused by FFN hT
        kT = P_act.tile([128, DS, S], bf, tag="kT")
        vv = P_act.tile([128, TT, H * (DH + 1)], bf, tag="vv")
        aoT = P_act.tile([128, DS, S], bf, tag="aoT")
        vsT = P_act.tile([128, S], bf, tag="vsT")
        tr_all = P_act.tile([128, TT, NB], f32, tag="tr")

        # ---- Q/K projections (outputs stay [d_out, tok]) ----
        def qk_chunk(qc):
            for ot in range(DS):
                for (w_l, dstT) in ((wq_l, qT), (wk_l, kT)):
                    ps = P_ps.tile([128, 512], f32, tag="ps")
                    for ds in range(DS):
                        nc.tensor.matmul(
                            ps, w_l[:, ds, ot * 128:(ot + 1) * 128],
                            nrmT[:, ds, qc * 512:(qc + 1) * 512],
                            start=(ds == 0), stop=(ds == DS - 1))
                    nc.scalar.copy(out=dstT[:, ot, qc * 512:(qc + 1) * 512],
                                   in_=ps)

        # ---- routing: fin -> top8 -> masked softmax -> token_recipe ----
        def routing_tile(t):
            fin_ps = P_ps.tile([128, 512], f32, tag="ps")
            for ds in range(DS):
                nc.tensor.matmul(fin_ps[:, :NN],
                                 nrmT[:, ds, t * 128:(t + 1) * 128],
                                 g_l[:, ds, :],
                                 start=(ds == 0), stop=(ds == DS - 1))
            fin = fin_ps[:, :NN]
            m8 = P_rt.tile([128, 8], f32, tag="m8")
            nc.vector.max(out=m8, in_=fin)
            t8 = P_sm.tile([128, 1], f32, tag="t8")
            nc.vector.reduce_sum(out=t8, in_=m8, axis=mybir.AxisListType.X,
                                 op=Alu.min)   # 8th largest, order-agnostic
            nt8 = P_sm.tile([128, 1], f32, tag="nt8")
            nc.vector.tensor_scalar_mul(out=nt8, in0=t8, scalar1=-1.0)
            er = P_rt.tile([128, NN], f32, tag="er")
            nc.scalar.activation(out=er, in_=fin, func=AF.Exp, bias=nt8)
            we = P_rt.tile([128, NN], f32, tag="we")
            nc.vector.scalar_tensor_tensor(out=we, in0=fin, scalar=t8,
                                           in1=er, op0=Alu.is_ge, op1=Alu.mult)
            dn = P_sm.tile([128, 1], f32, tag="dn")
            nc.vector.reduce_sum(out=dn, in_=we, axis=mybir.AxisListType.X)
            rc = P_sm.tile([128, 1], f32, tag="rc")
            nc.vector.reciprocal(out=rc, in_=dn)
            wfull = P_rt.tile([128, NN], bf, tag="wfull")
            nc.vector.tensor_scalar_mul(out=wfull, in0=we, scalar1=rc)
            wfT = P_rt.tile([128, 2, 128], bf, tag="wfT")
            for ns in range(2):
                transpose128(wfT[:, ns, :], wfull[:, ns * 128:(ns + 1) * 128])
            tr_ps = P_ps.tile([128, 512], f32, tag="ps")
            for ns in range(2):
                nc.tensor.matmul(tr_ps[:, :NB], wfT[:, ns, :], rec_l[:, ns, :],
                                 start=(ns == 0), stop=(ns == 1))
            nc.vector.tensor_copy(out=tr_all[:, t, :], in_=tr_ps[:, :NB])

        # LN1 (QK qc0 fills tiles 4-7); QK qc1 splits the routing DVE chain
        ln_pass(nrmT, mid=lambda: qk_chunk(0))
        for t in range(4):
            routing_tile(t)
        qk_chunk(1)
        for t in range(4, TT):
            routing_tile(t)

        # ---- xA (4 basis mats per matmul; A streamed in halves) ----
        # Weighted accumulation split across engines: first A-half through a
        # vector STT chain (acc_v), second half through gpsimd (acc_g),
        # merged by one vector add straight into the bf16 tile.
        a_halves = []
        for ah in range(2):
            a_l = P_ac.tile([128, DS, NB * R // 2], bf, tag="ac")
            nc.sync.dma_start(
                out=a_l,
                in_=dr["a_cat"][l][:, :, ah * (NB * R // 2):(ah + 1) * (NB * R // 2)])
            a_halves.append(a_l)

        def xa_tile(t):
            # Weighted accumulation split between a vector STT chain (from
            # PSUM) and scalar-engine scale-copies (activation Copy with
            # per-partition scale) into z slots + one strided vector reduce.
            # Tiles 0-3 run before attention qc0, so the scalar engine is
            # free and takes half the bases; tiles 4-7 overlap qc0's exps,
            # so scalar only takes a quarter.
            nz_act = 16
            acc_v = P_rt.tile([128, R], f32, tag="accv")
            zsl = P_rt.tile([128, 16, R], bf, tag="zsl", bufs=1)
            zj = 0
            first_v = True
            for ah in range(2):
                a_l = a_halves[ah]
                for g in range(NG // 2):
                    psA = P_psA.tile([128, 512], f32, tag="psA")
                    for ds in range(DS):
                        nc.tensor.matmul(psA,
                                         nrmT[:, ds, t * 128:(t + 1) * 128],
                                         a_l[:, ds, g * 512:(g + 1) * 512],
                                         start=(ds == 0), stop=(ds == DS - 1))
                    for ni in range(4):
                        n = ah * 16 + g * 4 + ni
                        to_act = (ni == 3) if nz_act == 8 else (ni >= 2)
                        if to_act:
                            nc.scalar.activation(
                                out=zsl[:, zj, :],
                                in_=psA[:, ni * R:(ni + 1) * R],
                                func=AF.Copy,
                                scale=tr_all[:, t, n:n + 1])
                            zj += 1
                        elif first_v:
                            nc.vector.tensor_scalar_mul(
                                out=acc_v, in0=psA[:, :R],
                                scalar1=tr_all[:, t, n:n + 1])
                            first_v = False
                        else:
                            nc.vector.scalar_tensor_tensor(
                                out=acc_v,
                                in0=psA[:, ni * R:(ni + 1) * R],
                                scalar=tr_all[:, t, n:n + 1],
                                in1=acc_v,
                                op0=Alu.mult, op1=Alu.add)
            acc_z = P_rt.tile([128, R], f32, tag="accz")
            nc.vector.tensor_reduce(
                out=acc_z, in_=zsl[:, 0:zj, :].rearrange("p n r -> p r n"),
                axis=mybir.AxisListType.X, op=Alu.add)
            vs_bf = P_rt.tile([128, R], bf, tag="vsbf")
            nc.vector.tensor_tensor(out=vs_bf, in0=acc_v, in1=acc_z,
                                    op=Alu.add)
            transpose128(vsT[:, t * 128:(t + 1) * 128], vs_bf)
            psv = P_ps.tile([128, 512], f32, tag="ps")
            nc.tensor.matmul(psv, vsT[:, t * 128:(t + 1) * 128], wv_l,
                             start=True, stop=True)
            # per-head layout [Vv_h | 1]: the ones column makes the AV matmul
            # also produce the softmax denominator (psum partition 64)
            vvh = vv[:, t, :].rearrange("p (h e) -> p h e", h=H)
            nc.scalar.copy(out=vvh[:, :, 0:DH],
                           in_=psv.rearrange("p (h e) -> p h e", h=H))
            nc.vector.memset(vvh[:, :, DH:DH + 1], 1.0)

        # ---- attention chunk (scoresT; exp/AV narrowed to causal cols) ----
        # Software-pipelined one head ahead: scores(h+1) are emitted before
        # AV(h) so the PE keeps streaming while the scalar engine exps head h.
        # `inter` supplies PE-heavy thunks interleaved between head pairs.
        def attn_chunk(qc, inter=None):
            nkt = qc * 4 + 4
            aoU8 = P_sm.tile([DH + 1, H, 512], bf, tag="aoU8", bufs=1)
            dn8 = P_sm.tile([98, 512], bf, tag="dn8", bufs=1)
            eTs = {}

            def scores_block(h):
                hp = (h % 2) * 64
                hd = h // 2
                eT = P_attn.tile([128, TT, 512], bf, tag="eT")
                eTs[h] = eT
                for kt in range(nkt):
                    kt_rel = kt - qc * 4
                    lo = max(0, kt_rel) * 128
                    pss = P_psS.tile([128, 512], f32, tag="psS")
                    nc.tensor.matmul(
                        pss[:, lo:512],
                        kT[hp:hp + 64, hd, kt * 128:(kt + 1) * 128],
                        qT[hp:hp + 64, hd, qc * 512 + lo:(qc + 1) * 512],
                        start=True, stop=True)
                    nc.scalar.activation(out=eT[:, kt, lo:512],
                                         in_=pss[:, lo:512], func=AF.Exp)
                    if kt_rel >= 0:
                        nc.gpsimd.affine_select(
                            out=eT[:, kt, kt_rel * 128:(kt_rel + 1) * 128],
                            in_=eT[:, kt, kt_rel * 128:(kt_rel + 1) * 128],
                            compare_op=Alu.is_ge, fill=0.0, base=0,
                            pattern=[[1, 128]], channel_multiplier=-1)

            def av_block(h):
                eT = eTs.pop(h)
                psa = P_psA.tile([128, 512], f32, tag="psA")
                for kt in range(nkt):
                    kt_rel = kt - qc * 4
                    lo = max(0, kt_rel) * 128
                    nc.tensor.matmul(
                        psa[0:DH + 1, lo:512],
                        vv[:, kt, h * (DH + 1):(h + 1) * (DH + 1)],
                        eT[:, kt, lo:512],
                        start=(kt == 0), stop=(kt == nkt - 1))
                # copy out of PSUM early (frees the bank); denom row (part 64)
                # is DMA'd onto its own partition of dn8 for a batched recip
                nc.vector.tensor_copy(out=aoU8[:, h, :], in_=psa[0:DH + 1, :])
                hrow = 32 * (h // 4) + h % 4
                nc.gpsimd.dma_start(out=dn8[hrow:hrow + 1, :],
                                    in_=aoU8[DH:DH + 1, h, :])

            # denominators handled per head-pair right after the pair's AV
            # blocks so the broadcast+divide overlaps later heads' work
            rec8b = P_sm.tile([98, 512], bf, tag="rec8b", bufs=1)

            def denom_quad(q):
                pb = 32 * q
                with nc.allow_low_precision(reason="attn denom recip bf16"):
                    nc.vector.reciprocal(out=rec8b[pb:pb + 4, :],
                                         in_=dn8[pb:pb + 4, :])
                for h in range(4 * q, 4 * q + 4):
                    hp = (h % 2) * 64
                    hd = h // 2
                    rb = P_psS.tile([64, 512], f32, tag="psS")
                    nc.tensor.matmul(rb, sel[pb:pb + 4, h * 64:(h + 1) * 64],
                                     rec8b[pb:pb + 4, :],
                                     start=True, stop=True)
                    if hp == 0:
                        nc.vector.tensor_tensor(
                            out=aoT[0:64, hd, qc * 512:(qc + 1) * 512],
                            in0=aoU8[0:DH, h, :], in1=rb, op=Alu.mult)
                    else:
                        tmp = P_sm.tile([64, 512], bf, tag="aotmp", bufs=1)
                        nc.vector.tensor_tensor(out=tmp, in0=aoU8[0:DH, h, :],
                                                in1=rb, op=Alu.mult)
                        nc.sync.dma_start(
                            out=aoT[64:128, hd, qc * 512:(qc + 1) * 512],
                            in_=tmp)

            for h in range(H):
                if inter and h % 2 == 0:
                    inter.pop(0)()
                scores_block(h)
                if h >= 2:
                    av_block(h - 2)
                    if h == 5:
                        denom_quad(0)
            while inter:
                inter.pop(0)()
            av_block(H - 2)
            av_block(H - 1)
            denom_quad(1)

        # ---- attention out projection + residual for one token tile ----
        def ao_proj(t):
            pso = P_ps.tile([128, 512], f32, tag="ps")
            for ds in range(DS):
                nc.tensor.matmul(pso, aoT[:, ds, t * 128:(t + 1) * 128],
                                 wao_l[:, ds, :],
                                 start=(ds == 0), stop=(ds == DS - 1))
            nc.vector.tensor_tensor(out=x_sb[:, t, :], in0=pso,
                                    in1=x_sb[:, t, :], op=Alu.add)

        # interleave: xA tiles 4-7 fill qc0's exp gaps; ao-proj of the first
        # token tiles fills qc1's exp gaps
        for t in range(4):
            xa_tile(t)
        attn_chunk(0, inter=[lambda tt=t: xa_tile(4 + tt) for t in range(4)])
        attn_chunk(1, inter=[lambda tt=t: ao_proj(tt) for t in range(4)])
        for t in range(4, TT):
            ao_proj(t)

        # ---- FFN (up qc0 overlaps LN2 tiles 4-7) ----
        wup_l = P_big.tile([128, DS, DFF], bf, tag="big")
        wdn_l = P_big.tile([128, FS, D], bf, tag="big")
        nc.sync.dma_start(out=wup_l, in_=dr["wup"][l])
        nc.sync.dma_start(out=wdn_l, in_=dr["wdn"][l])
        n2T = P_act.tile([128, DS, S], bf, tag="nrmT")  # nrmT is dead by now
        hTs = {}

        def ffn_up(qc):
            hT = P_act.tile([128, FS, 512], bf, tag="qT")
            hTs[qc] = hT
            for ft in range(FS):
                psu = P_ps.tile([128, 512], f32, tag="ps")
                for ds in range(DS):
                    nc.tensor.matmul(psu,
                                     wup_l[:, ds, ft * 128:(ft + 1) * 128],
                                     n2T[:, ds, qc * 512:(qc + 1) * 512],
                                     start=(ds == 0), stop=(ds == DS - 1))
                nc.scalar.activation(out=hT[:, ft, :], in_=psu, func=AF.Gelu)

        def ffn_down(qc):
            hT = hTs[qc]
            for tr in range(4):
                t = qc * 4 + tr
                psd = P_ps.tile([128, 512], f32, tag="ps")
                for fs in range(FS):
                    nc.tensor.matmul(psd, hT[:, fs, tr * 128:(tr + 1) * 128],
                                     wdn_l[:, fs, :],
                                     start=(fs == 0), stop=(fs == FS - 1))
                nc.vector.tensor_tensor(out=x_sb[:, t, :], in0=psd,
                                        in1=x_sb[:, t, :], op=Alu.add)

        ln_pass(n2T, mid=lambda: ffn_up(0))
        ffn_down(0)
        ffn_up(1)
        ffn_down(1)

    # ---- final LN + tied head over this core's vocab half ----
    # Out-DMAs go through the gpsimd queue so the sync queue carries only
    # emb prefetches (keeps them ahead of compute).
    xfT = P_act.tile([128, DS, S], bf, tag="nrmT")
    ln_pass(xfT)
    for vg in range(VG):
        emb = P_big.tile([128, DS, VG * VPG * VCH // VG], bf, tag="big")
        nc.sync.dma_start(
            out=emb, in_=dr["eT"][:, :, vg * VPG * VCH:(vg + 1) * VPG * VCH])
        for t in range(TT):
            for vi in range(VPG):
                psh = P_ps.tile([128, 512], f32, tag="ps")
                for ds in range(DS):
                    nc.tensor.matmul(
                        psh[:, :VCH], xfT[:, ds, t * 128:(t + 1) * 128],
                        emb[:, ds, vi * VCH:(vi + 1) * VCH],
                        start=(ds == 0), stop=(ds == DS - 1))
                stage = P_hd.tile([128, VCH], bf, tag="stage")
                if vi % 2 == 0:
                    nc.vector.tensor_copy(out=stage, in_=psh[:, :VCH])
                else:
                    nc.scalar.copy(out=stage, in_=psh[:, :VCH])
                off = (vg * VPG + vi) * VCH
                nc.gpsimd.dma_start(
                    out=dr["out"][t * 128:(t + 1) * 128, off:off + VCH],
                    in_=stage)
    ctx.close()


def kernel(**inputs):
    from concourse.bass_utils import run_bass_kernel_spmd

    if "nc" not in _cache:
        _cache["nc"] = _build_nc()
    nc = _cache["nc"]

    in_maps = _preprocess(inputs)
    res = run_bass_kernel_spmd(nc, in_maps, core_ids=list(range(NCORES)))
    global _last_results
    _last_results = res.results

    out = np.empty((B, S, VOC), dtype=np.float32)
    for c in range(NCORES):
        b, half = c // 2, c % 2
        out[b, :, half * VH:(half + 1) * VH] = \
            res.results[c]["logits"].astype(np.float32)
    return out



# revision 64
# speedup vs baseline: 1.0005x; 1.0005x over previous
"""Trainium2 Bass kernel for nn_DAWN_10419590660472 (moe_routing transformer).

Sharding: 8 cores = 4 batches x 2 vocab-halves. Each core computes the full
4-layer body for its batch (the causal residual stream cannot be
sequence-split without communication), then the tied-embedding head for all
1024 tokens over its 16000-entry vocab half. All cores run the SAME program;
only input data differs. Heavy matmuls in bf16 with fp32 PSUM accumulation.

Host-folded math:
- attn.mean(-1) == 1/S exactly (softmax rows sum to 1), so the routing gate
  sigmoid(ctx) is a per-layer constant folded into the sem projection.
- LN affine params fold into downstream weights (identity here: s=1, b=0).
- top_k(fin, 8): nc.vector.max yields the 8 largest per token; token_recipe
  = masked_softmax(fin) @ rec_sm, a matmul (no gather).
- Attention softmax needs no max-subtraction (scores are O(1)); denominators
  come free from a ones-augmented value matrix in the AV matmul and are
  folded into the PSUM->SBUF copy as a reciprocal multiply.
"""

import numpy as np
import ml_dtypes

VOC = 32000; D = 512; DFF = 2048; L = 4; H = 8; DH = D // H
NN = 256; NB = 32; R = 128; B = 4; S = 1024
NCORES = 8
TT = S // 128          # 8 token tiles
DS = D // 128          # 4 d-slices
FS = DFF // 128        # 16 dff-slices
QC = S // 512          # 2 query chunks
NG = NB // 4           # 8 groups of 4 basis matrices
VH = VOC // 2          # vocab half per core
VCH = 500              # head vocab chunk (<=512)
NVC = VH // VCH        # 32
VG = 8                 # emb streaming groups
VPG = NVC // VG        # 4 chunks per group
EPS = 1e-5

BF16 = ml_dtypes.bfloat16
_cache = {}
DEBUG = False


def _softmax_np(x, axis=-1):
    m = x.max(axis=axis, keepdims=True)
    e = np.exp(x - m)
    return e / e.sum(axis=axis, keepdims=True)


def _preprocess(inputs):
    f32 = lambda k: np.asarray(inputs[k], dtype=np.float32)
    ids = np.asarray(inputs["input_ids"])
    token_emb = f32("token_emb"); pos_emb = f32("pos_emb")
    basis_A = f32("basis_A"); basis_emb = f32("basis_emb")
    q_w = f32("q_w"); k_w = f32("k_w"); ao_w = f32("ao_w")
    recipe = f32("recipe"); ctx_pat = f32("ctx_pat")
    vout_w = f32("vout_w"); up_w = f32("up_w"); down_w = f32("down_w")
    ln1_s = f32("ln1_s"); ln2_s = f32("ln2_s"); lnf_s = f32("lnf_s")

    for k in ("q_b", "k_b", "ao_b", "vout_b", "up_b", "down_b",
              "ln1_b", "ln2_b", "lnf_b"):
        assert not np.any(np.asarray(inputs[k])), f"nonzero {k} unsupported"

    scale = 1.0 / np.sqrt(DH)
    x0 = token_emb[ids] + pos_emb[:S][None]              # [B, S, D]

    def part_first(a, nslice):
        # [nslice*128, F] -> [128, nslice, F]
        return np.ascontiguousarray(
            a.reshape(nslice, 128, -1).transpose(1, 0, 2))

    wq = np.empty((L, 128, DS, D), dtype=BF16)
    wk = np.empty((L, 128, DS, D), dtype=BF16)
    wao = np.empty((L, 128, DS, D), dtype=BF16)
    gT = np.empty((L, 128, DS, NN), dtype=BF16)
    recT = np.empty((L, 128, 2, NB), dtype=BF16)
    a_cat = np.empty((L, 128, DS, NB * R), dtype=BF16)
    wvout = np.empty((L, 128, D), dtype=BF16)
    wup = np.empty((L, 128, DS, DFF), dtype=BF16)
    wdn = np.empty((L, 128, FS, D), dtype=BF16)

    for l in range(L):
        wq[l] = part_first((q_w[l] * ln1_s[l][None, :] * scale).T, DS)
        wk[l] = part_first((k_w[l] * ln1_s[l][None, :]).T, DS)
        wao[l] = part_first(ao_w[l].T, DS)
        rs = _softmax_np(recipe[l])                      # [NN, NB]
        emb_sem = rs @ basis_emb                         # [NN, D]
        gate = 1.0 / (1.0 + np.exp(-(ctx_pat[l].sum(-1) / S)))
        gT[l] = part_first(((emb_sem * ln1_s[l][None, :]) * gate[:, None]).T, DS)
        recT[l] = part_first(rs, 2)
        ae = basis_A * ln1_s[l][None, :, None]           # [NB, D, R]
        a_cat[l] = part_first(ae.transpose(1, 0, 2).reshape(D, NB * R), DS)
        wvout[l] = vout_w[l].T.astype(BF16)              # [R, D]
        wup[l] = part_first((up_w[l] * ln2_s[l][None, :]).T, DS)
        wdn[l] = part_first(down_w[l].T, FS)

    eT_full = part_first((token_emb * lnf_s[None, :]).T, DS).astype(BF16)
    ident = np.eye(128, dtype=BF16)
    # head h's ones-row lives at partition 32*(h//4) + h%4 so that per-quad
    # reciprocal/broadcast ops start at 32-aligned partitions
    sel = np.zeros((128, H * 64), dtype=BF16)
    for h in range(H):
        sel[32 * (h // 4) + h % 4, h * 64:(h + 1) * 64] = 1

    shared = dict(wq=wq, wk=wk, wao=wao, gT=gT, recT=recT, a_cat=a_cat,
                  wvout=wvout, wup=wup, wdn=wdn, ident=ident, sel=sel)
    per_core = []
    for c in range(NCORES):
        b, half = c // 2, c % 2
        m = dict(shared)
        m["x0"] = np.ascontiguousarray(x0[b]).astype(np.float32)
        m["eT"] = np.ascontiguousarray(eT_full[:, :, half * VH:(half + 1) * VH])
        per_core.append(m)
    return per_core


def _build_nc():
    import concourse.mybir as mybir
    import concourse.tile as tile
    from concourse import bacc
    from concourse.alu_op_type import AluOpType as Alu

    AF = mybir.ActivationFunctionType
    bf = mybir.dt.bfloat16
    f32 = mybir.dt.float32

    nc = bacc.Bacc("TRN2", target_bir_lowering=False, debug=False,
                   num_devices=NCORES)

    din = lambda n, shp, dt=bf: nc.dram_tensor(n, shp, dt, kind="ExternalInput")
    dr = dict(
        x0=din("x0", [S, D], f32),
        wq=din("wq", [L, 128, DS, D]), wk=din("wk", [L, 128, DS, D]),
        wao=din("wao", [L, 128, DS, D]), gT=din("gT", [L, 128, DS, NN]),
        recT=din("recT", [L, 128, 2, NB]),
        a_cat=din("a_cat", [L, 128, DS, NB * R]),
        wvout=din("wvout", [L, 128, D]), wup=din("wup", [L, 128, DS, DFF]),
        wdn=din("wdn", [L, 128, FS, D]), eT=din("eT", [128, DS, VH]),
        ident=din("ident", [128, 128]), sel=din("sel", [128, H * 64]),
        out=nc.dram_tensor("logits", [S, VH], bf, kind="ExternalOutput"),
    )

    with tile.TileContext(nc) as tc:
        _emit(nc, tc, mybir, Alu, AF, bf, f32, dr)

    nc.compile()
    return nc


def _emit(nc, tc, mybir, Alu, AF, bf, f32, dr):
    from contextlib import ExitStack
    ctx = ExitStack()
    pool = lambda name, bufs, space="SBUF": ctx.enter_context(
        tc.tile_pool(name=name, bufs=bufs, space=space))

    P_x = pool("x", 1)
    P_const = pool("const", 1)
    P_w = pool("w", 2)                 # small per-layer weights (double-buffered)
    P_ac = pool("ac", 2)               # a_cat halves
    P_big = pool("big", 2)             # wup / wdn / emb chunks
    P_act = pool("act", 1)             # per-layer activations
    P_nrm = pool("nrm", 1)             # token-major LN outputs (transient)
    P_attn = pool("attn", 3)           # eT buffers (2-ahead score pipeline)
    P_rt = pool("rt", 2)               # routing temporaries
    P_sm = pool("sm", 2)               # small stats tiles
    P_hd = pool("hd", 6)               # head staging
    P_ps = pool("ps", 2, "PSUM")       # generic matmul psum
    P_psT = pool("psT", 2, "PSUM")     # transpose psum
    P_psA = pool("psA", 2, "PSUM")     # xA / AV psum
    P_psS = pool("psS", 2, "PSUM")     # scores psum + denom broadcast

    ident = P_const.tile([128, 128], bf)
    nc.sync.dma_start(out=ident, in_=dr["ident"][:, :])
    eps_sb = P_const.tile([128, 1], f32)
    nc.vector.memset(eps_sb, EPS)
    # sel[:, h*64:(h+1)*64] is all-ones on partition h, zero elsewhere:
    # matmul(lhsT=sel_h, rhs=rec8[0:8]) broadcasts rec8 row h to 64 partitions
    sel = P_const.tile([128, H * 64], bf)
    nc.sync.dma_start(out=sel, in_=dr["sel"][:, :])

    x_sb = P_x.tile([128, TT, D], f32)
    for t in range(TT):
        nc.sync.dma_start(out=x_sb[:, t, :],
                          in_=dr["x0"][t * 128:(t + 1) * 128, :])

    def ln_pass(dstT, mid=None):
        # batched stats for all 8 token tiles -> one sqrt + one reciprocal
        mv8 = P_sm.tile([128, TT, 2], f32, tag="mv8")
        for t in range(TT):
            stats = P_sm.tile([128, 6], f32, tag="st")
            nc.vector.bn_stats(out=stats, in_=x_sb[:, t, :])
            nc.vector.bn_aggr(out=mv8[:, t, :], in_=stats)
        rstd8 = P_sm.tile([128, TT], f32, tag="rs8")
        nc.scalar.activation(out=rstd8, in_=mv8[:, :, 1], func=AF.Sqrt,
                             bias=eps_sb)
        nc.vector.reciprocal(out=rstd8, in_=rstd8)
        for t in range(TT):
            nrm = P_nrm.tile([128, D], bf, tag="nrm")
            nc.vector.tensor_scalar(out=nrm, in0=x_sb[:, t, :],
                                    scalar1=mv8[:, t, 0:1],
                                    scalar2=rstd8[:, t:t + 1],
                                    op0=Alu.subtract, op1=Alu.mult)
            for ds in range(DS):
                transpose128(dstT[:, ds, t * 128:(t + 1) * 128],
                             nrm[:, ds * 128:(ds + 1) * 128],
                             cp=(nc.vector if ds % 2 == 0 else nc.scalar))
            if t == 3 and mid is not None:
                mid()   # interleave PE-heavy work behind tiles 4-7

    def transpose128(dst_sb, src_sb, cp=None):
        ps = P_psT.tile([128, 128], bf)
        nc.tensor.transpose(ps, src_sb, ident)
        if cp is nc.scalar:
            nc.scalar.copy(out=dst_sb, in_=ps)
        else:
            nc.vector.tensor_copy(out=dst_sb, in_=ps)

    for l in range(L):
        wq_l = P_w.tile([128, DS, D], bf, tag="wq", bufs=1)
        wk_l = P_w.tile([128, DS, D], bf, tag="wk", bufs=1)
        wao_l = P_w.tile([128, DS, D], bf, tag="wao", bufs=1)
        g_l = P_w.tile([128, DS, NN], bf, tag="g", bufs=1)
        rec_l = P_w.tile([128, 2, NB], bf, tag="rec")
        wv_l = P_w.tile([128, D], bf, tag="wv", bufs=1)
        nc.sync.dma_start(out=wq_l, in_=dr["wq"][l])
        nc.sync.dma_start(out=wk_l, in_=dr["wk"][l])
        nc.sync.dma_start(out=wao_l, in_=dr["wao"][l])
        nc.sync.dma_start(out=g_l, in_=dr["gT"][l])
        nc.sync.dma_start(out=rec_l, in_=dr["recT"][l])
        nc.sync.dma_start(out=wv_l, in_=dr["wvout"][l])

        nrmT = P_act.tile([128, DS, S], bf, tag="nrmT")
        qT = P_act.tile([128, DS, S], bf, tag="qT")  # slot reused by FFN hT
        kT = P_act.tile([128, DS, S], bf, tag="kT")
        vv = P_act.tile([128, TT, H * (DH + 1)], bf, tag="vv")
        aoT = P_act.tile([128, DS, S], bf, tag="aoT")
        vsT = P_act.tile([128, S], bf, tag="vsT")
        tr_all = P_act.tile([128, TT, NB], f32, tag="tr")

        # ---- Q/K projections (outputs stay [d_out, tok]) ----
        def qk_chunk(qc):
            for ot in range(DS):
                for (w_l, dstT) in ((wq_l, qT), (wk_l, kT)):
                    ps = P_ps.tile([128, 512], f32, tag="ps")
                    for ds in range(DS):
                        nc.tensor.matmul(
                            ps, w_l[:, ds, ot * 128:(ot + 1) * 128],
                            nrmT[:, ds, qc * 512:(qc + 1) * 512],
                            start=(ds == 0), stop=(ds == DS - 1))
                    nc.scalar.copy(out=dstT[:, ot, qc * 512:(qc + 1) * 512],
                                   in_=ps)

        # ---- routing: fin -> top8 -> masked softmax -> token_recipe ----
        def routing_tile(t):
            fin_ps = P_ps.tile([128, 512], f32, tag="ps")
            for ds in range(DS):
                nc.tensor.matmul(fin_ps[:, :NN],
                                 nrmT[:, ds, t * 128:(t + 1) * 128],
                                 g_l[:, ds, :],
                                 start=(ds == 0), stop=(ds == DS - 1))
            fin = fin_ps[:, :NN]
            m8 = P_rt.tile([128, 8], f32, tag="m8")
            nc.vector.max(out=m8, in_=fin)
            t8 = P_sm.tile([128, 1], f32, tag="t8")
            nc.vector.reduce_sum(out=t8, in_=m8, axis=mybir.AxisListType.X,
                                 op=Alu.min)   # 8th largest, order-agnostic
            nt8 = P_sm.tile([128, 1], f32, tag="nt8")
            nc.vector.tensor_scalar_mul(out=nt8, in0=t8, scalar1=-1.0)
            er = P_rt.tile([128, NN], f32, tag="er")
            nc.scalar.activation(out=er, in_=fin, func=AF.Exp, bias=nt8)
            we = P_rt.tile([128, NN], f32, tag="we")
            nc.vector.scalar_tensor_tensor(out=we, in0=fin, scalar=t8,
                                           in1=er, op0=Alu.is_ge, op1=Alu.mult)
            dn = P_sm.tile([128, 1], f32, tag="dn")
            nc.vector.reduce_sum(out=dn, in_=we, axis=mybir.AxisListType.X)
            rc = P_sm.tile([128, 1], f32, tag="rc")
            nc.vector.reciprocal(out=rc, in_=dn)
            wfull = P_rt.tile([128, NN], bf, tag="wfull")
            nc.vector.tensor_scalar_mul(out=wfull, in0=we, scalar1=rc)
            wfT = P_rt.tile([128, 2, 128], bf, tag="wfT")
            for ns in range(2):
                transpose128(wfT[:, ns, :], wfull[:, ns * 128:(ns + 1) * 128])
            tr_ps = P_ps.tile([128, 512], f32, tag="ps")
            for ns in range(2):
                nc.tensor.matmul(tr_ps[:, :NB], wfT[:, ns, :], rec_l[:, ns, :],
                                 start=(ns == 0), stop=(ns == 1))
            nc.vector.tensor_copy(out=tr_all[:, t, :], in_=tr_ps[:, :NB])

        # LN1 (QK qc0 fills tiles 4-7); QK qc1 splits the routing DVE chain
        ln_pass(nrmT, mid=lambda: qk_chunk(0))
        for t in range(4):
            routing_tile(t)
        qk_chunk(1)
        for t in range(4, TT):
            routing_tile(t)

        # ---- xA (4 basis mats per matmul; A streamed in halves) ----
        # Weighted accumulation split across engines: first A-half through a
        # vector STT chain (acc_v), second half through gpsimd (acc_g),
        # merged by one vector add straight into the bf16 tile.
        a_halves = []
        for ah in range(2):
            a_l = P_ac.tile([128, DS, NB * R // 2], bf, tag="ac")
            nc.sync.dma_start(
                out=a_l,
                in_=dr["a_cat"][l][:, :, ah * (NB * R // 2):(ah + 1) * (NB * R // 2)])
            a_halves.append(a_l)

        def xa_tile(t):
            # Weighted accumulation split between a vector STT chain (from
            # PSUM) and scalar-engine scale-copies (activation Copy with
            # per-partition scale) into z slots + one strided vector reduce.
            # Tiles 0-3 run before attention qc0, so the scalar engine is
            # free and takes half the bases; tiles 4-7 overlap qc0's exps,
            # so scalar only takes a quarter.
            nz_act = 16 if t < 4 else 8
            acc_v = P_rt.tile([128, R], f32, tag="accv")
            zsl = P_rt.tile([128, 16, R], bf, tag="zsl", bufs=1)
            zj = 0
            first_v = True
            for ah in range(2):
                a_l = a_halves[ah]
                for g in range(NG // 2):
                    psA = P_psA.tile([128, 512], f32, tag="psA")
                    for ds in range(DS):
                        nc.tensor.matmul(psA,
                                         nrmT[:, ds, t * 128:(t + 1) * 128],
                                         a_l[:, ds, g * 512:(g + 1) * 512],
                                         start=(ds == 0), stop=(ds == DS - 1))
                    for ni in range(4):
                        n = ah * 16 + g * 4 + ni
                        to_act = (ni == 3) if nz_act == 8 else (ni >= 2)
                        if to_act:
                            nc.scalar.activation(
                                out=zsl[:, zj, :],
                                in_=psA[:, ni * R:(ni + 1) * R],
                                func=AF.Copy,
                                scale=tr_all[:, t, n:n + 1])
                            zj += 1
                        elif first_v:
                            nc.vector.tensor_scalar_mul(
                                out=acc_v, in0=psA[:, :R],
                                scalar1=tr_all[:, t, n:n + 1])
                            first_v = False
                        else:
                            nc.vector.scalar_tensor_tensor(
                                out=acc_v,
                                in0=psA[:, ni * R:(ni + 1) * R],
                                scalar=tr_all[:, t, n:n + 1],
                                in1=acc_v,
                                op0=Alu.mult, op1=Alu.add)
            acc_z = P_rt.tile([128, R], f32, tag="accz")
            nc.vector.tensor_reduce(
                out=acc_z, in_=zsl[:, 0:zj, :].rearrange("p n r -> p r n"),
                axis=mybir.AxisListType.X, op=Alu.add)
            vs_bf = P_rt.tile([128, R], bf, tag="vsbf")
            nc.vector.tensor_tensor(out=vs_bf, in0=acc_v, in1=acc_z,
                                    op=Alu.add)
            transpose128(vsT[:, t * 128:(t + 1) * 128], vs_bf)
            psv = P_ps.tile([128, 512], f32, tag="ps")
            nc.tensor.matmul(psv, vsT[:, t * 128:(t + 1) * 128], wv_l,
                             start=True, stop=True)
            # per-head layout [Vv_h | 1]: the ones column makes the AV matmul
            # also produce the softmax denominator (psum partition 64)
            vvh = vv[:, t, :].rearrange("p (h e) -> p h e", h=H)
            nc.scalar.copy(out=vvh[:, :, 0:DH],
                           in_=psv.rearrange("p (h e) -> p h e", h=H))
            nc.vector.memset(vvh[:, :, DH:DH + 1], 1.0)

        # ---- attention chunk (scoresT; exp/AV narrowed to causal cols) ----
        # Software-pipelined one head ahead: scores(h+1) are emitted before
        # AV(h) so the PE keeps streaming while the scalar engine exps head h.
        # `inter` supplies PE-heavy thunks interleaved between head pairs.
        def attn_chunk(qc, inter=None):
            nkt = qc * 4 + 4
            aoU8 = P_sm.tile([DH + 1, H, 512], bf, tag="aoU8", bufs=1)
            dn8 = P_sm.tile([98, 512], bf, tag="dn8", bufs=1)
            eTs = {}

            def scores_block(h):
                hp = (h % 2) * 64
                hd = h // 2
                eT = P_attn.tile([128, TT, 512], bf, tag="eT")
                eTs[h] = eT
                for kt in range(nkt):
                    kt_rel = kt - qc * 4
                    lo = max(0, kt_rel) * 128
                    pss = P_psS.tile([128, 512], f32, tag="psS")
                    nc.tensor.matmul(
                        pss[:, lo:512],
                        kT[hp:hp + 64, hd, kt * 128:(kt + 1) * 128],
                        qT[hp:hp + 64, hd, qc * 512 + lo:(qc + 1) * 512],
                        start=True, stop=True)
                    nc.scalar.activation(out=eT[:, kt, lo:512],
                                         in_=pss[:, lo:512], func=AF.Exp)
                    if kt_rel >= 0:
                        nc.gpsimd.affine_select(
                            out=eT[:, kt, kt_rel * 128:(kt_rel + 1) * 128],
                            in_=eT[:, kt, kt_rel * 128:(kt_rel + 1) * 128],
                            compare_op=Alu.is_ge, fill=0.0, base=0,
                            pattern=[[1, 128]], channel_multiplier=-1)

            def av_block(h):
                eT = eTs.pop(h)
                psa = P_psA.tile([128, 512], f32, tag="psA")
                for kt in range(nkt):
                    kt_rel = kt - qc * 4
                    lo = max(0, kt_rel) * 128
                    nc.tensor.matmul(
                        psa[0:DH + 1, lo:512],
                        vv[:, kt, h * (DH + 1):(h + 1) * (DH + 1)],
                        eT[:, kt, lo:512],
                        start=(kt == 0), stop=(kt == nkt - 1))
                # copy out of PSUM early (frees the bank); denom row (part 64)
                # is DMA'd onto its own partition of dn8 for a batched recip
                nc.vector.tensor_copy(out=aoU8[:, h, :], in_=psa[0:DH + 1, :])
                hrow = 32 * (h // 4) + h % 4
                nc.gpsimd.dma_start(out=dn8[hrow:hrow + 1, :],
                                    in_=aoU8[DH:DH + 1, h, :])

            # denominators handled per head-pair right after the pair's AV
            # blocks so the broadcast+divide overlaps later heads' work
            rec8b = P_sm.tile([98, 512], bf, tag="rec8b", bufs=1)

            def denom_quad(q):
                pb = 32 * q
                with nc.allow_low_precision(reason="attn denom recip bf16"):
                    nc.vector.reciprocal(out=rec8b[pb:pb + 4, :],
                                         in_=dn8[pb:pb + 4, :])
                for h in range(4 * q, 4 * q + 4):
                    hp = (h % 2) * 64
                    hd = h // 2
                    rb = P_psS.tile([64, 512], f32, tag="psS")
                    nc.tensor.matmul(rb, sel[pb:pb + 4, h * 64:(h + 1) * 64],
                                     rec8b[pb:pb + 4, :],
                                     start=True, stop=True)
                    if hp == 0:
                        nc.vector.tensor_tensor(
                            out=aoT[0:64, hd, qc * 512:(qc + 1) * 512],
                            in0=aoU8[0:DH, h, :], in1=rb, op=Alu.mult)
                    else:
                        tmp = P_sm.tile([64, 512], bf, tag="aotmp", bufs=1)
                        nc.vector.tensor_tensor(out=tmp, in0=aoU8[0:DH, h, :],
                                                in1=rb, op=Alu.mult)
                        nc.sync.dma_start(
                            out=aoT[64:128, hd, qc * 512:(qc + 1) * 512],
                            in_=tmp)

            for h in range(H):
                if inter and h % 2 == 0:
                    inter.pop(0)()
                scores_block(h)
                if h >= 2:
                    av_block(h - 2)
                    if h == 5:
                        denom_quad(0)
            while inter:
                inter.pop(0)()
            av_block(H - 2)
            av_block(H - 1)
            denom_quad(1)

        # ---- attention out projection + residual for one token tile ----
        def ao_proj(t):
            pso = P_ps.tile([128, 512], f32, tag="ps")
            for ds in range(DS):
                nc.tensor.matmul(pso, aoT[:, ds, t * 128:(t + 1) * 128],
                                 wao_l[:, ds, :],
                                 start=(ds == 0), stop=(ds == DS - 1))
            nc.vector.tensor_tensor(out=x_sb[:, t, :], in0=pso,
                                    in1=x_sb[:, t, :], op=Alu.add)

        # interleave: xA tiles 4-7 fill qc0's exp gaps; ao-proj of the first
        # token tiles fills qc1's exp gaps
        for t in range(4):
            xa_tile(t)
        attn_chunk(0, inter=[lambda tt=t: xa_tile(4 + tt) for t in range(4)])
        attn_chunk(1, inter=[lambda tt=t: ao_proj(tt) for t in range(4)])
        for t in range(4, TT):
            ao_proj(t)

        # ---- FFN (up qc0 overlaps LN2 tiles 4-7) ----
        wup_l = P_big.tile([128, DS, DFF], bf, tag="big")
        wdn_l = P_big.tile([128, FS, D], bf, tag="big")
        nc.sync.dma_start(out=wup_l, in_=dr["wup"][l])
        nc.sync.dma_start(out=wdn_l, in_=dr["wdn"][l])
        n2T = P_act.tile([128, DS, S], bf, tag="nrmT")  # nrmT is dead by now
        hTs = {}

        def ffn_up(qc):
            hT = P_act.tile([128, FS, 512], bf, tag="qT")
            hTs[qc] = hT
            for ft in range(FS):
                psu = P_ps.tile([128, 512], f32, tag="ps")
                for ds in range(DS):
                    nc.tensor.matmul(psu,
                                     wup_l[:, ds, ft * 128:(ft + 1) * 128],
                                     n2T[:, ds, qc * 512:(qc + 1) * 512],
                                     start=(ds == 0), stop=(ds == DS - 1))
                nc.scalar.activation(out=hT[:, ft, :], in_=psu, func=AF.Gelu)

        def ffn_down(qc):
            hT = hTs[qc]
            for tr in range(4):
                t = qc * 4 + tr
                psd = P_ps.tile([128, 512], f32, tag="ps")
                for fs in range(FS):
                    nc.tensor.matmul(psd, hT[:, fs, tr * 128:(tr + 1) * 128],
                                     wdn_l[:, fs, :],
                                     start=(fs == 0), stop=(fs == FS - 1))
                nc.vector.tensor_tensor(out=x_sb[:, t, :], in0=psd,
                                        in1=x_sb[:, t, :], op=Alu.add)

        ln_pass(n2T, mid=lambda: ffn_up(0))
        ffn_down(0)
        ffn_up(1)
        ffn_down(1)

    # ---- final LN + tied head over this core's vocab half ----
    # Out-DMAs go through the gpsimd queue so the sync queue carries only
    # emb prefetches (keeps them ahead of compute).
    xfT = P_act.tile([128, DS, S], bf, tag="nrmT")
    ln_pass(xfT)
    for vg in range(VG):
        emb = P_big.tile([128, DS, VG * VPG * VCH // VG], bf, tag="big")
        nc.sync.dma_start(
            out=emb, in_=dr["eT"][:, :, vg * VPG * VCH:(vg + 1) * VPG * VCH])
        for t in range(TT):
            for vi in range(VPG):
                psh = P_ps.tile([128, 512], f32, tag="ps")
                for ds in range(DS):
                    nc.tensor.matmul(
                        psh[:, :VCH], xfT[:, ds, t * 128:(t + 1) * 128],
                        emb[:, ds, vi * VCH:(vi + 1) * VCH],
                        start=(ds == 0), stop=(ds == DS - 1))
                stage = P_hd.tile([128, VCH], bf, tag="stage")
                if vi % 2 == 0:
                    nc.vector.tensor_copy(out=stage, in_=psh[:, :VCH])
                else:
                    nc.scalar.copy(out=stage, in_=psh[:, :VCH])
                off = (vg * VPG + vi) * VCH
                nc.gpsimd.dma_start(
                    out=dr["out"][t * 128:(t + 1) * 128, off:off + VCH],
                    in_=stage)
    ctx.close()


def kernel(**inputs):
    from concourse.bass_utils import run_bass_kernel_spmd

    if "nc" not in _cache:
        _cache["nc"] = _build_nc()
    nc = _cache["nc"]

    in_maps = _preprocess(inputs)
    res = run_bass_kernel_spmd(nc, in_maps, core_ids=list(range(NCORES)))
    global _last_results
    _last_results = res.results

    out = np.empty((B, S, VOC), dtype=np.float32)
    for c in range(NCORES):
        b, half = c // 2, c % 2
        out[b, :, half * VH:(half + 1) * VH] = \
            res.results[c]["logits"].astype(np.float32)
    return out



# revision 71
# speedup vs baseline: 1.0111x; 1.0106x over previous
"""Trainium2 Bass kernel for nn_DAWN_10419590660472 (moe_routing transformer).

Sharding: 8 cores = 4 batches x 2 vocab-halves. Each core computes the full
4-layer body for its batch (the causal residual stream cannot be
sequence-split without communication), then the tied-embedding head for all
1024 tokens over its 16000-entry vocab half. All cores run the SAME program;
only input data differs. Heavy matmuls in bf16 with fp32 PSUM accumulation.

Host-folded math:
- attn.mean(-1) == 1/S exactly (softmax rows sum to 1), so the routing gate
  sigmoid(ctx) is a per-layer constant folded into the sem projection.
- LN affine params fold into downstream weights (identity here: s=1, b=0).
- top_k(fin, 8): nc.vector.max yields the 8 largest per token; token_recipe
  = masked_softmax(fin) @ rec_sm, a matmul (no gather).
- Attention softmax needs no max-subtraction (scores are O(1)); denominators
  come free from a ones-augmented value matrix in the AV matmul and are
  folded into the PSUM->SBUF copy as a reciprocal multiply.
"""

import numpy as np
import ml_dtypes

VOC = 32000; D = 512; DFF = 2048; L = 4; H = 8; DH = D // H
NN = 256; NB = 32; R = 128; B = 4; S = 1024
NCORES = 8
TT = S // 128          # 8 token tiles
DS = D // 128          # 4 d-slices
FS = DFF // 128        # 16 dff-slices
QC = S // 512          # 2 query chunks
NG = NB // 4           # 8 groups of 4 basis matrices
VH = VOC // 2          # vocab half per core
VCH = 500              # head vocab chunk (<=512)
NVC = VH // VCH        # 32
VG = 8                 # emb streaming groups
VPG = NVC // VG        # 4 chunks per group
EPS = 1e-5

BF16 = ml_dtypes.bfloat16
_cache = {}
DEBUG = False


def _softmax_np(x, axis=-1):
    m = x.max(axis=axis, keepdims=True)
    e = np.exp(x - m)
    return e / e.sum(axis=axis, keepdims=True)


def _preprocess(inputs):
    f32 = lambda k: np.asarray(inputs[k], dtype=np.float32)
    ids = np.asarray(inputs["input_ids"])
    token_emb = f32("token_emb"); pos_emb = f32("pos_emb")
    basis_A = f32("basis_A"); basis_emb = f32("basis_emb")
    q_w = f32("q_w"); k_w = f32("k_w"); ao_w = f32("ao_w")
    recipe = f32("recipe"); ctx_pat = f32("ctx_pat")
    vout_w = f32("vout_w"); up_w = f32("up_w"); down_w = f32("down_w")
    ln1_s = f32("ln1_s"); ln2_s = f32("ln2_s"); lnf_s = f32("lnf_s")

    for k in ("q_b", "k_b", "ao_b", "vout_b", "up_b", "down_b",
              "ln1_b", "ln2_b", "lnf_b"):
        assert not np.any(np.asarray(inputs[k])), f"nonzero {k} unsupported"

    scale = 1.0 / np.sqrt(DH)
    x0 = token_emb[ids] + pos_emb[:S][None]              # [B, S, D]

    def part_first(a, nslice):
        # [nslice*128, F] -> [128, nslice, F]
        return np.ascontiguousarray(
            a.reshape(nslice, 128, -1).transpose(1, 0, 2))

    wq = np.empty((L, 128, DS, D), dtype=BF16)
    wk = np.empty((L, 128, DS, D), dtype=BF16)
    wao = np.empty((L, 128, DS, D), dtype=BF16)
    gT = np.empty((L, 128, DS, NN), dtype=BF16)
    recT = np.empty((L, 128, 2, NB), dtype=BF16)
    a_cat = np.empty((L, 128, DS, NB * R), dtype=BF16)
    wvout = np.empty((L, 128, D), dtype=BF16)
    wup = np.empty((L, 128, DS, DFF), dtype=BF16)
    wdn = np.empty((L, 128, FS, D), dtype=BF16)

    for l in range(L):
        wq[l] = part_first((q_w[l] * ln1_s[l][None, :] * scale).T, DS)
        wk[l] = part_first((k_w[l] * ln1_s[l][None, :]).T, DS)
        wao[l] = part_first(ao_w[l].T, DS)
        rs = _softmax_np(recipe[l])                      # [NN, NB]
        emb_sem = rs @ basis_emb                         # [NN, D]
        gate = 1.0 / (1.0 + np.exp(-(ctx_pat[l].sum(-1) / S)))
        gT[l] = part_first(((emb_sem * ln1_s[l][None, :]) * gate[:, None]).T, DS)
        recT[l] = part_first(rs, 2)
        ae = basis_A * ln1_s[l][None, :, None]           # [NB, D, R]
        a_cat[l] = part_first(ae.transpose(1, 0, 2).reshape(D, NB * R), DS)
        wvout[l] = vout_w[l].T.astype(BF16)              # [R, D]
        wup[l] = part_first((up_w[l] * ln2_s[l][None, :]).T, DS)
        wdn[l] = part_first(down_w[l].T, FS)

    eT_full = part_first((token_emb * lnf_s[None, :]).T, DS).astype(BF16)
    ident = np.eye(128, dtype=BF16)
    sel = np.zeros((8, H * 64), dtype=BF16)
    for h in range(H):
        sel[h, h * 64:(h + 1) * 64] = 1

    shared = dict(wq=wq, wk=wk, wao=wao, gT=gT, recT=recT, a_cat=a_cat,
                  wvout=wvout, wup=wup, wdn=wdn, ident=ident, sel=sel)
    per_core = []
    for c in range(NCORES):
        b, half = c // 2, c % 2
        m = dict(shared)
        m["x0"] = np.ascontiguousarray(x0[b]).astype(np.float32)
        m["eT"] = np.ascontiguousarray(eT_full[:, :, half * VH:(half + 1) * VH])
        per_core.append(m)
    return per_core


def _build_nc():
    import concourse.mybir as mybir
    import concourse.tile as tile
    from concourse import bacc
    from concourse.alu_op_type import AluOpType as Alu

    AF = mybir.ActivationFunctionType
    bf = mybir.dt.bfloat16
    f32 = mybir.dt.float32

    nc = bacc.Bacc("TRN2", target_bir_lowering=False, debug=False,
                   num_devices=NCORES)

    din = lambda n, shp, dt=bf: nc.dram_tensor(n, shp, dt, kind="ExternalInput")
    dr = dict(
        x0=din("x0", [S, D], f32),
        wq=din("wq", [L, 128, DS, D]), wk=din("wk", [L, 128, DS, D]),
        wao=din("wao", [L, 128, DS, D]), gT=din("gT", [L, 128, DS, NN]),
        recT=din("recT", [L, 128, 2, NB]),
        a_cat=din("a_cat", [L, 128, DS, NB * R]),
        wvout=din("wvout", [L, 128, D]), wup=din("wup", [L, 128, DS, DFF]),
        wdn=din("wdn", [L, 128, FS, D]), eT=din("eT", [128, DS, VH]),
        ident=din("ident", [128, 128]), sel=din("sel", [8, H * 64]),
        out=nc.dram_tensor("logits", [S, VH], bf, kind="ExternalOutput"),
    )

    with tile.TileContext(nc) as tc:
        _emit(nc, tc, mybir, Alu, AF, bf, f32, dr)

    nc.compile()
    return nc


def _emit(nc, tc, mybir, Alu, AF, bf, f32, dr):
    from contextlib import ExitStack
    ctx = ExitStack()
    pool = lambda name, bufs, space="SBUF": ctx.enter_context(
        tc.tile_pool(name=name, bufs=bufs, space=space))

    P_x = pool("x", 1)
    P_const = pool("const", 1)
    P_w = pool("w", 2)                 # small per-layer weights (double-buffered)
    P_ac = pool("ac", 2)               # a_cat halves
    P_big = pool("big", 2)             # wup / wdn / emb chunks
    P_act = pool("act", 1)             # per-layer activations
    P_nrm = pool("nrm", 1)             # token-major LN outputs (transient)
    P_attn = pool("attn", 3)           # eT buffers (2-ahead score pipeline)
    P_rt = pool("rt", 2)               # routing temporaries
    P_sm = pool("sm", 2)               # small stats tiles
    P_hd = pool("hd", 6)               # head staging
    P_ps = pool("ps", 2, "PSUM")       # generic matmul psum
    P_psT = pool("psT", 1, "PSUM")     # transpose psum
    P_psA = pool("psA", 2, "PSUM")     # xA / AV psum
    P_psS = pool("psS", 3, "PSUM")     # scores psum + denom broadcast

    ident = P_const.tile([128, 128], bf)
    nc.sync.dma_start(out=ident, in_=dr["ident"][:, :])
    eps_sb = P_const.tile([128, 1], f32)
    nc.vector.memset(eps_sb, EPS)
    # sel[:, h*64:(h+1)*64] is all-ones on partition h, zero elsewhere:
    # matmul(lhsT=sel_h, rhs=rec8[0:8]) broadcasts rec8 row h to 64 partitions
    sel = P_const.tile([8, H * 64], bf)
    nc.sync.dma_start(out=sel, in_=dr["sel"][:, :])

    x_sb = P_x.tile([128, TT, D], f32)
    for t in range(TT):
        nc.sync.dma_start(out=x_sb[:, t, :],
                          in_=dr["x0"][t * 128:(t + 1) * 128, :])

    def ln_pass(dstT, mid=None):
        # batched stats for all 8 token tiles -> one sqrt + one reciprocal
        mv8 = P_sm.tile([128, TT, 2], f32, tag="mv8")
        for t in range(TT):
            stats = P_sm.tile([128, 6], f32, tag="st")
            nc.vector.bn_stats(out=stats, in_=x_sb[:, t, :])
            nc.vector.bn_aggr(out=mv8[:, t, :], in_=stats)
        rstd8 = P_sm.tile([128, TT], f32, tag="rs8")
        nc.scalar.activation(out=rstd8, in_=mv8[:, :, 1], func=AF.Sqrt,
                             bias=eps_sb)
        nc.vector.reciprocal(out=rstd8, in_=rstd8)
        for t in range(TT):
            nrm = P_nrm.tile([128, D], bf, tag="nrm")
            nc.vector.tensor_scalar(out=nrm, in0=x_sb[:, t, :],
                                    scalar1=mv8[:, t, 0:1],
                                    scalar2=rstd8[:, t:t + 1],
                                    op0=Alu.subtract, op1=Alu.mult)
            for ds in range(DS):
                transpose128(dstT[:, ds, t * 128:(t + 1) * 128],
                             nrm[:, ds * 128:(ds + 1) * 128],
                             cp=(nc.vector if ds % 2 == 0 else nc.scalar))
            if t == 3 and mid is not None:
                mid()   # interleave PE-heavy work behind tiles 4-7

    def transpose128(dst_sb, src_sb, cp=None):
        ps = P_psT.tile([128, 128], bf)
        nc.tensor.transpose(ps, src_sb, ident)
        if cp is nc.scalar:
            nc.scalar.copy(out=dst_sb, in_=ps)
        else:
            nc.vector.tensor_copy(out=dst_sb, in_=ps)

    for l in range(L):
        wq_l = P_w.tile([128, DS, D], bf, tag="wq", bufs=1)
        wk_l = P_w.tile([128, DS, D], bf, tag="wk", bufs=1)
        wao_l = P_w.tile([128, DS, D], bf, tag="wao", bufs=1)
        g_l = P_w.tile([128, DS, NN], bf, tag="g", bufs=1)
        rec_l = P_w.tile([128, 2, NB], bf, tag="rec")
        wv_l = P_w.tile([128, D], bf, tag="wv", bufs=1)
        nc.sync.dma_start(out=wq_l, in_=dr["wq"][l])
        nc.sync.dma_start(out=wk_l, in_=dr["wk"][l])
        nc.sync.dma_start(out=wao_l, in_=dr["wao"][l])
        nc.sync.dma_start(out=g_l, in_=dr["gT"][l])
        nc.sync.dma_start(out=rec_l, in_=dr["recT"][l])
        nc.sync.dma_start(out=wv_l, in_=dr["wvout"][l])

        nrmT = P_act.tile([128, DS, S], bf, tag="nrmT")
        qT = P_act.tile([128, DS, S], bf, tag="qT")  # slot reused by FFN hT
        kT = P_act.tile([128, DS, S], bf, tag="kT")
        vv = P_act.tile([128, TT, H * (DH + 1)], bf, tag="vv")
        aoT = P_act.tile([128, DS, S], bf, tag="aoT")
        vsT = P_act.tile([128, S], bf, tag="vsT")
        tr_all = P_act.tile([128, TT, NB], f32, tag="tr")

        # ---- Q/K projections (outputs stay [d_out, tok]) ----
        def qk_chunk(qc):
            for ot in range(DS):
                for (w_l, dstT) in ((wq_l, qT), (wk_l, kT)):
                    ps = P_ps.tile([128, 512], f32, tag="ps")
                    for ds in range(DS):
                        nc.tensor.matmul(
                            ps, w_l[:, ds, ot * 128:(ot + 1) * 128],
                            nrmT[:, ds, qc * 512:(qc + 1) * 512],
                            start=(ds == 0), stop=(ds == DS - 1))
                    nc.scalar.copy(out=dstT[:, ot, qc * 512:(qc + 1) * 512],
                                   in_=ps)

        # ---- routing: fin -> top8 -> masked softmax -> token_recipe ----
        def routing_tile(t):
            fin_ps = P_ps.tile([128, 512], f32, tag="ps")
            for ds in range(DS):
                nc.tensor.matmul(fin_ps[:, :NN],
                                 nrmT[:, ds, t * 128:(t + 1) * 128],
                                 g_l[:, ds, :],
                                 start=(ds == 0), stop=(ds == DS - 1))
            fin = fin_ps[:, :NN]
            m8 = P_rt.tile([128, 8], f32, tag="m8")
            nc.vector.max(out=m8, in_=fin)
            t8 = P_sm.tile([128, 1], f32, tag="t8")
            nc.vector.reduce_sum(out=t8, in_=m8, axis=mybir.AxisListType.X,
                                 op=Alu.min)   # 8th largest, order-agnostic
            nt8 = P_sm.tile([128, 1], f32, tag="nt8")
            nc.vector.tensor_scalar_mul(out=nt8, in0=t8, scalar1=-1.0)
            er = P_rt.tile([128, NN], f32, tag="er")
            nc.scalar.activation(out=er, in_=fin, func=AF.Exp, bias=nt8)
            we = P_rt.tile([128, NN], f32, tag="we")
            nc.vector.scalar_tensor_tensor(out=we, in0=fin, scalar=t8,
                                           in1=er, op0=Alu.is_ge, op1=Alu.mult)
            dn = P_sm.tile([128, 1], f32, tag="dn")
            nc.vector.reduce_sum(out=dn, in_=we, axis=mybir.AxisListType.X)
            rc = P_sm.tile([128, 1], f32, tag="rc")
            nc.vector.reciprocal(out=rc, in_=dn)
            wfull = P_rt.tile([128, NN], bf, tag="wfull")
            nc.vector.tensor_scalar_mul(out=wfull, in0=we, scalar1=rc)
            wfT = P_rt.tile([128, 2, 128], bf, tag="wfT")
            for ns in range(2):
                transpose128(wfT[:, ns, :], wfull[:, ns * 128:(ns + 1) * 128])
            tr_ps = P_ps.tile([128, 512], f32, tag="ps")
            for ns in range(2):
                nc.tensor.matmul(tr_ps[:, :NB], wfT[:, ns, :], rec_l[:, ns, :],
                                 start=(ns == 0), stop=(ns == 1))
            nc.vector.tensor_copy(out=tr_all[:, t, :], in_=tr_ps[:, :NB])

        # LN1 (QK qc0 fills tiles 4-7); QK qc1 splits the routing DVE chain
        ln_pass(nrmT, mid=lambda: qk_chunk(0))
        for t in range(4):
            routing_tile(t)
        qk_chunk(1)
        for t in range(4, TT):
            routing_tile(t)

        # ---- xA (4 basis mats per matmul; A streamed in halves) ----
        # Weighted accumulation split across engines: first A-half through a
        # vector STT chain (acc_v), second half through gpsimd (acc_g),
        # merged by one vector add straight into the bf16 tile.
        a_halves = []
        for ah in range(2):
            a_l = P_ac.tile([128, DS, NB * R // 2], bf, tag="ac")
            nc.sync.dma_start(
                out=a_l,
                in_=dr["a_cat"][l][:, :, ah * (NB * R // 2):(ah + 1) * (NB * R // 2)])
            a_halves.append(a_l)

        def xa_tile(t):
            # Weighted accumulation split between a vector STT chain (from
            # PSUM) and scalar-engine scale-copies (activation Copy with
            # per-partition scale) into z slots + one strided vector reduce.
            # Tiles 0-3 run before attention qc0, so the scalar engine is
            # free and takes half the bases; tiles 4-7 overlap qc0's exps,
            # so scalar only takes a quarter.
            nz_act = 16 if t < 4 else 8
            acc_v = P_rt.tile([128, R], f32, tag="accv")
            zsl = P_rt.tile([128, 16, R], bf, tag="zsl", bufs=1)
            zj = 0
            first_v = True
            for ah in range(2):
                a_l = a_halves[ah]
                for g in range(NG // 2):
                    psA = P_psA.tile([128, 512], f32, tag="psA")
                    for ds in range(DS):
                        nc.tensor.matmul(psA,
                                         nrmT[:, ds, t * 128:(t + 1) * 128],
                                         a_l[:, ds, g * 512:(g + 1) * 512],
                                         start=(ds == 0), stop=(ds == DS - 1))
                    for ni in range(4):
                        n = ah * 16 + g * 4 + ni
                        to_act = (ni == 3) if nz_act == 8 else (ni >= 2)
                        if to_act:
                            nc.scalar.activation(
                                out=zsl[:, zj, :],
                                in_=psA[:, ni * R:(ni + 1) * R],
                                func=AF.Copy,
                                scale=tr_all[:, t, n:n + 1])
                            zj += 1
                        elif first_v:
                            nc.vector.tensor_scalar_mul(
                                out=acc_v, in0=psA[:, :R],
                                scalar1=tr_all[:, t, n:n + 1])
                            first_v = False
                        else:
                            nc.vector.scalar_tensor_tensor(
                                out=acc_v,
                                in0=psA[:, ni * R:(ni + 1) * R],
                                scalar=tr_all[:, t, n:n + 1],
                                in1=acc_v,
                                op0=Alu.mult, op1=Alu.add)
            acc_z = P_rt.tile([128, R], f32, tag="accz")
            nc.vector.tensor_reduce(
                out=acc_z, in_=zsl[:, 0:zj, :].rearrange("p n r -> p r n"),
                axis=mybir.AxisListType.X, op=Alu.add)
            vs_bf = P_rt.tile([128, R], bf, tag="vsbf")
            nc.vector.tensor_tensor(out=vs_bf, in0=acc_v, in1=acc_z,
                                    op=Alu.add)
            transpose128(vsT[:, t * 128:(t + 1) * 128], vs_bf)
            psv = P_ps.tile([128, 512], f32, tag="ps")
            nc.tensor.matmul(psv, vsT[:, t * 128:(t + 1) * 128], wv_l,
                             start=True, stop=True)
            # per-head layout [Vv_h | 1]: the ones column makes the AV matmul
            # also produce the softmax denominator (psum partition 64)
            vvh = vv[:, t, :].rearrange("p (h e) -> p h e", h=H)
            nc.scalar.copy(out=vvh[:, :, 0:DH],
                           in_=psv.rearrange("p (h e) -> p h e", h=H))
            nc.vector.memset(vvh[:, :, DH:DH + 1], 1.0)

        # ---- attention chunk (scoresT; exp/AV narrowed to causal cols) ----
        # Software-pipelined one head ahead: scores(h+1) are emitted before
        # AV(h) so the PE keeps streaming while the scalar engine exps head h.
        # `inter` supplies PE-heavy thunks interleaved between head pairs.
        def attn_chunk(qc, inter=None):
            nkt = qc * 4 + 4
            aoU8 = P_sm.tile([DH + 1, H, 512], bf, tag="aoU8", bufs=1)
            dn8 = P_sm.tile([H, 512], bf, tag="dn8", bufs=1)
            eTs = {}

            def scores_block(h):
                hp = (h % 2) * 64
                hd = h // 2
                eT = P_attn.tile([128, TT, 512], bf, tag="eT")
                eTs[h] = eT
                for kt in range(nkt):
                    kt_rel = kt - qc * 4
                    lo = max(0, kt_rel) * 128
                    pss = P_psS.tile([128, 512], f32, tag="psS")
                    nc.tensor.matmul(
                        pss[:, lo:512],
                        kT[hp:hp + 64, hd, kt * 128:(kt + 1) * 128],
                        qT[hp:hp + 64, hd, qc * 512 + lo:(qc + 1) * 512],
                        start=True, stop=True)
                    nc.scalar.activation(out=eT[:, kt, lo:512],
                                         in_=pss[:, lo:512], func=AF.Exp)
                    if kt_rel >= 0:
                        nc.gpsimd.affine_select(
                            out=eT[:, kt, kt_rel * 128:(kt_rel + 1) * 128],
                            in_=eT[:, kt, kt_rel * 128:(kt_rel + 1) * 128],
                            compare_op=Alu.is_ge, fill=0.0, base=0,
                            pattern=[[1, 128]], channel_multiplier=-1)

            def av_block(h):
                eT = eTs.pop(h)
                psa = P_psA.tile([128, 512], f32, tag="psA")
                for kt in range(nkt):
                    kt_rel = kt - qc * 4
                    lo = max(0, kt_rel) * 128
                    nc.tensor.matmul(
                        psa[0:DH + 1, lo:512],
                        vv[:, kt, h * (DH + 1):(h + 1) * (DH + 1)],
                        eT[:, kt, lo:512],
                        start=(kt == 0), stop=(kt == nkt - 1))
                # copy out of PSUM early (frees the bank); denom row (part 64)
                # is DMA'd onto its own partition of dn8 for a batched recip
                nc.vector.tensor_copy(out=aoU8[:, h, :], in_=psa[0:DH + 1, :])
                nc.gpsimd.dma_start(out=dn8[h:h + 1, :],
                                    in_=aoU8[DH:DH + 1, h, :])

            for h in range(H):
                if inter and h % 2 == 0:
                    inter.pop(0)()
                scores_block(h)
                if h >= 2:
                    av_block(h - 2)
            while inter:
                inter.pop(0)()
            av_block(H - 2)
            av_block(H - 1)
            # one reciprocal for all 8 heads (free-dim bound op), then
            # broadcast each row across 64 partitions via a PE rank-1 matmul
            rec8b = P_sm.tile([H, 512], bf, tag="rec8b", bufs=1)
            with nc.allow_low_precision(reason="attn denom recip to bf16"):
                nc.vector.reciprocal(out=rec8b, in_=dn8)
            for h in range(H):
                hp = (h % 2) * 64
                hd = h // 2
                rb = P_psS.tile([64, 512], f32, tag="psS")
                nc.tensor.matmul(rb, sel[:, h * 64:(h + 1) * 64],
                                 rec8b[0:8, :], start=True, stop=True)
                if hp == 0:
                    nc.vector.tensor_tensor(
                        out=aoT[0:64, hd, qc * 512:(qc + 1) * 512],
                        in0=aoU8[0:DH, h, :], in1=rb, op=Alu.mult)
                else:
                    tmp = P_sm.tile([64, 512], bf, tag="aotmp", bufs=1)
                    nc.vector.tensor_tensor(out=tmp, in0=aoU8[0:DH, h, :],
                                            in1=rb, op=Alu.mult)
                    nc.sync.dma_start(
                        out=aoT[64:128, hd, qc * 512:(qc + 1) * 512], in_=tmp)

        # ---- attention out projection + residual for one token tile ----
        def ao_proj(t):
            pso = P_ps.tile([128, 512], f32, tag="ps")
            for ds in range(DS):
                nc.tensor.matmul(pso, aoT[:, ds, t * 128:(t + 1) * 128],
                                 wao_l[:, ds, :],
                                 start=(ds == 0), stop=(ds == DS - 1))
            nc.vector.tensor_tensor(out=x_sb[:, t, :], in0=pso,
                                    in1=x_sb[:, t, :], op=Alu.add)

        # interleave: xA tiles 4-7 fill qc0's exp gaps; ao-proj of the first
        # token tiles fills qc1's exp gaps
        for t in range(4):
            xa_tile(t)
        attn_chunk(0, inter=[lambda tt=t: xa_tile(4 + tt) for t in range(4)])
        attn_chunk(1, inter=[lambda tt=t: ao_proj(tt) for t in range(4)])
        for t in range(4, TT):
            ao_proj(t)

        # ---- FFN (up qc0 overlaps LN2 tiles 4-7) ----
        wup_l = P_big.tile([128, DS, DFF], bf, tag="big")
        wdn_l = P_big.tile([128, FS, D], bf, tag="big")
        nc.sync.dma_start(out=wup_l, in_=dr["wup"][l])
        nc.sync.dma_start(out=wdn_l, in_=dr["wdn"][l])
        n2T = P_act.tile([128, DS, S], bf, tag="nrmT")  # nrmT is dead by now
        hTs = {}

        def ffn_up(qc):
            hT = P_act.tile([128, FS, 512], bf, tag="qT")
            hTs[qc] = hT
            for ft in range(FS):
                psu = P_ps.tile([128, 512], f32, tag="ps")
                for ds in range(DS):
                    nc.tensor.matmul(psu,
                                     wup_l[:, ds, ft * 128:(ft + 1) * 128],
                                     n2T[:, ds, qc * 512:(qc + 1) * 512],
                                     start=(ds == 0), stop=(ds == DS - 1))
                nc.scalar.activation(out=hT[:, ft, :], in_=psu, func=AF.Gelu)

        def ffn_down(qc):
            hT = hTs[qc]
            for tr in range(4):
                t = qc * 4 + tr
                psd = P_ps.tile([128, 512], f32, tag="ps")
                for fs in range(FS):
                    nc.tensor.matmul(psd, hT[:, fs, tr * 128:(tr + 1) * 128],
                                     wdn_l[:, fs, :],
                                     start=(fs == 0), stop=(fs == FS - 1))
                nc.vector.tensor_tensor(out=x_sb[:, t, :], in0=psd,
                                        in1=x_sb[:, t, :], op=Alu.add)

        ln_pass(n2T, mid=lambda: ffn_up(0))
        ffn_down(0)
        ffn_up(1)
        ffn_down(1)

    # ---- final LN + tied head over this core's vocab half ----
    # Out-DMAs go through the gpsimd queue so the sync queue carries only
    # emb prefetches (keeps them ahead of compute).
    xfT = P_act.tile([128, DS, S], bf, tag="nrmT")
    ln_pass(xfT)
    for vg in range(VG):
        emb = P_big.tile([128, DS, VG * VPG * VCH // VG], bf, tag="big")
        nc.sync.dma_start(
            out=emb, in_=dr["eT"][:, :, vg * VPG * VCH:(vg + 1) * VPG * VCH])
        for t in range(TT):
            for vi in range(VPG):
                psh = P_ps.tile([128, 512], f32, tag="ps")
                for ds in range(DS):
                    nc.tensor.matmul(
                        psh[:, :VCH], xfT[:, ds, t * 128:(t + 1) * 128],
                        emb[:, ds, vi * VCH:(vi + 1) * VCH],
                        start=(ds == 0), stop=(ds == DS - 1))
                stage = P_hd.tile([128, VCH], bf, tag="stage")
                if vi % 2 == 0:
                    nc.vector.tensor_copy(out=stage, in_=psh[:, :VCH])
                else:
                    nc.scalar.copy(out=stage, in_=psh[:, :VCH])
                off = (vg * VPG + vi) * VCH
                nc.gpsimd.dma_start(
                    out=dr["out"][t * 128:(t + 1) * 128, off:off + VCH],
                    in_=stage)
    ctx.close()


def kernel(**inputs):
    from concourse.bass_utils import run_bass_kernel_spmd

    if "nc" not in _cache:
        _cache["nc"] = _build_nc()
    nc = _cache["nc"]

    in_maps = _preprocess(inputs)
    res = run_bass_kernel_spmd(nc, in_maps, core_ids=list(range(NCORES)))
    global _last_results
    _last_results = res.results

    out = np.empty((B, S, VOC), dtype=np.float32)
    for c in range(NCORES):
        b, half = c // 2, c % 2
        out[b, :, half * VH:(half + 1) * VH] = \
            res.results[c]["logits"].astype(np.float32)
    return out

